# revision 58
# baseline (speedup 1.0000x reference)
"""GATv2 layer kernel for Trainium2 (Bass/Tile), 8-core SPMD.

Problem (hardcoded): B=4, N=512, D=128, H=8 heads, F=16 hidden, is_concat.
  g_l = h @ W_l.T ; g_r = h @ W_r.T               [B,N,H,F]
  e[b,i,j,h] = sum_f a_w[f]*lrelu(g_l[b,j,h,f] + g_r[b,i,h,f], 0.2)
  a = softmax_j(e masked by adj)                  [B,i,j,H]
  out[b,i,h,f] = sum_j a[b,i,j,h]*g_r[b,j,h,f]   -> [B,N,H*F]

Sharding: 8 cores = (batch b in 0..4) x (i-half in 0..2). Each core handles
256 target nodes i of one batch with fully-replicated g_l/g_r.

Math used on device (per core, b fixed):
  lrelu(x) = 0.8*relu(x) + 0.2*x, so
  e[i,j,h] = 0.8*sum_hf A[hf,h]*relu(g_lT[hf,j] + g_rT[hf,i]) + 0.2*alT[h,j]
             + 0.2*ar[i,h]
  The per-(i,h) additive term 0.2*ar cancels in softmax (shift invariance),
  so it is dropped. Masking is multiplicative on exp(e) (exact zeros).
  Softmax normalization is applied after aggregation (linearity).

Layouts (per group of 16 i's, partitions = (i_local*8 + h)):
  t[hf, j]      = relu(g_lT + g_rT[:, i] bias)        ACT/DVE
  psum[(i,h),j] = A_aw.T @ t  (M=8 stripes)           PE
  psum += 0.2*alT via an extra accumulating matmul   PE
  u             = exp(psum)                           ACT (reads PSUM)
  um, den       = u * mask_rep, rowsum                DVE (fused)
  umT           = transpose(um) (4x 128x128)          PE
  aggT[(i,h),hf]= sum_j umT.T @ g_r                   PE (4 K-chunks)
  agg_sb        = aggT * (1/den) * headmask           DVE (fused)
  out[i,hf]     = R.T @ agg_sb  (sum over h)          PE
"""

import ml_dtypes
import numpy as np
from contextlib import ExitStack

import concourse.bass as bass
import concourse.bacc as bacc
import concourse.tile as tile
import concourse.mybir as mybir
from concourse.bass_utils import run_bass_kernel_spmd

B, N, D = 4, 512, 128
H, F = 8, 16
NEG_SLOPE = 0.2
NCORES = 8
IHALF = N // 2          # 256 target nodes per core
GSIZE = 16              # i's per group
NGROUPS = IHALF // GSIZE  # 16
f32 = mybir.dt.float32
f16 = mybir.dt.float16

# The score path (relu'd pairwise features t and the per-node score matmuls)
# runs in fp16: full 1-cycle/row PE streaming, DVE packed 16-bit modes, and
# fast-weight-load with background-buffer overlap -- with a 10-bit mantissa
# (TF32-class, ~5e-4) and ample range for these tiny values. The noise lands
# only in pre-softmax scores; softmax normalization and the aggregation path
# (attention weights x g_r and the output) stay exact fp32.

# How many of the 16 per-group relu ops go to DVE (rest on ScalarE).
RELU_ON_DVE = 12


def build_program():
    nc = bacc.Bacc(
        "TRN2", target_bir_lowering=False, debug=False, num_devices=NCORES
    )

    d_hT = nc.dram_tensor("hT", [D, N], f16, kind="ExternalInput").ap()
    d_WlT = nc.dram_tensor("WlT", [D, H * F], f16, kind="ExternalInput").ap()
    d_WrT = nc.dram_tensor("WrT", [D, H * F], f16, kind="ExternalInput").ap()
    # Amask[:, 64k:64k+64] is 0.8*A_aw placed in the (k%8)-th 8-column block
    # of a [128, 64] half-array stationary operand (zeros elsewhere): 16
    # accumulating M=64 matmuls (8 per 64-row half, alternating PE column
    # strips so LDWEIGHTS overlaps the other half's in-flight matmul) compose
    # 16 target nodes into one full-height PSUM tile. lrelu = 0.8*relu +
    # 0.2*identity; the 0.8 is folded into these weights.
    d_Aaw = nc.dram_tensor("Amask", [H * F, 120], f16, kind="ExternalInput").ap()
    d_Arep = nc.dram_tensor("Arep02", [H * F, 128], f16, kind="ExternalInput").ap()
    # Rmask[:, 64q:64q+64] holds the head-sum reduction matrix placed in
    # columns [16q:16q+16] (4 accumulating matmuls -> one 64-row PSUM stripe).
    d_R = nc.dram_tensor("Rmask", [128, 4 * 64], f16, kind="ExternalInput").ap()
    d_hm = nc.dram_tensor("headmask", [128, H * F], f32, kind="ExternalInput").ap()
    d_id = nc.dram_tensor("ident", [128, 128], f16, kind="ExternalInput").ap()
    d_mask = nc.dram_tensor("maskseg", [IHALF, N], f16, kind="ExternalInput").ap()
    d_out = nc.dram_tensor("out", [IHALF, D], f32, kind="ExternalOutput").ap()

    with tile.TileContext(nc) as tc:
        with ExitStack() as ctx:
            _gat_body(ctx, tc, d_out, d_hT, d_WlT, d_WrT, d_Aaw, d_Arep,
                      d_R, d_hm, d_id, d_mask)
    nc.compile()
    return nc


def _gat_body(ctx, tc, d_out, d_hT, d_WlT, d_WrT, d_Aaw, d_Arep, d_R, d_hm,
              d_id, d_mask):
    nc = tc.nc
    add = mybir.AluOpType.add
    mult = mybir.AluOpType.mult
    amax = mybir.AluOpType.max
    Relu = mybir.ActivationFunctionType.Relu
    Exp = mybir.ActivationFunctionType.Exp

    consts = ctx.enter_context(tc.tile_pool(name="consts", bufs=1))
    tpool = ctx.enter_context(tc.tile_pool(name="tpool", bufs=12))
    upool = ctx.enter_context(tc.tile_pool(name="upool", bufs=3))
    umpool = ctx.enter_context(tc.tile_pool(name="umpool", bufs=4))
    maskp = ctx.enter_context(tc.tile_pool(name="maskp", bufs=4))
    umtp = ctx.enter_context(tc.tile_pool(name="umtp", bufs=3))
    aggp = ctx.enter_context(tc.tile_pool(name="aggp", bufs=3))
    denp = ctx.enter_context(tc.tile_pool(name="denp", bufs=3))
    outp = ctx.enter_context(tc.tile_pool(name="outp", bufs=2))

    ppe = ctx.enter_context(tc.tile_pool(name="ppe", bufs=3, space="PSUM"))
    pumt = ctx.enter_context(tc.tile_pool(name="pumt", bufs=2, space="PSUM"))
    pagg = ctx.enter_context(tc.tile_pool(name="pagg", bufs=2, space="PSUM"))
    pout = ctx.enter_context(tc.tile_pool(name="pout", bufs=1, space="PSUM"))

    # ---- load constants ----
    s_WlT = consts.tile([D, H * F], f16, tag="wlt")
    nc.sync.dma_start(out=s_WlT[:], in_=d_WlT)
    s_WrT = consts.tile([D, H * F], f16, tag="wrt")
    nc.sync.dma_start(out=s_WrT[:], in_=d_WrT)
    s_hT = consts.tile([D, N], f16, tag="ht")
    nc.sync.dma_start(out=s_hT[:], in_=d_hT)
    s_Aaw = consts.tile([H * F, 120], f16, tag="aaw")
    nc.gpsimd.dma_start(out=s_Aaw[:], in_=d_Aaw)
    s_Arep = consts.tile([H * F, 128], f16, tag="arep")
    nc.gpsimd.dma_start(out=s_Arep[:], in_=d_Arep)
    s_R = consts.tile([128, 4 * 64], f16, tag="rmat")
    nc.gpsimd.dma_start(out=s_R[:], in_=d_R)
    s_hm = consts.tile([128, H * F], f32, tag="hm")
    nc.gpsimd.dma_start(out=s_hm[:], in_=d_hm)
    s_id = consts.tile([128, 128], f16, tag="ident")
    nc.gpsimd.dma_start(out=s_id[:], in_=d_id)

    # ---- setup: projections ----
    # g_lT[hf, j] = sum_d WlT[d, hf] * hT[d, j]  (kept in bf16: feeds the
    # bf16 score path only)
    g_lT = consts.tile([H * F, N], f16, tag="glt")
    ps = ppe.tile([128, N], f32, tag="pe")
    nc.tensor.matmul(ps[:], s_WlT[:], s_hT[:], start=True, stop=True)
    nc.scalar.copy(g_lT[:], ps[:])

    g_rT = consts.tile([H * F, N], f32, tag="grt")
    ps = ppe.tile([128, N], f32, tag="pe")
    nc.tensor.matmul(ps[:], s_WrT[:], s_hT[:], start=True, stop=True)
    nc.scalar.copy(g_rT[:], ps[:])

    # g_r natural layout with an appended ones column per 128-row chunk:
    # chunk c occupies cols [129c, 129c+129); col 129c+128 is all-ones so the
    # aggregation matmul also produces the softmax denominator for free.
    g_r_nat = consts.tile([128, 4 * 129], f16, tag="grnat")
    for c in range(4):
        cs = slice(128 * c, 128 * (c + 1))
        pq = pagg.tile([128, 129], f32, tag="agg")
        nc.tensor.matmul(pq[:, 0:128], s_hT[:, cs], s_WrT[:],
                         start=True, stop=True)
        nc.vector.tensor_copy(g_r_nat[:, 129 * c:129 * c + 128], pq[:, 0:128])
        nc.vector.memset(g_r_nat[:, 129 * c + 128:129 * c + 129], 1.0)

    # The 0.2*alT linear term is accumulated into each group's score PSUM by
    # an extra matmul (lhsT=s_Arep, rhs=g_lT) -- no materialized alT tile.

    # ---- main loop: pairs of 16-node groups ----
    # The transpose/aggregation stage is batched per pair: one umT PSUM bank
    # (fp16 [128,1024] fits a 2KiB bank), one PSUM->SBUF copy, one paired
    # aggregation PSUM tile [128,258] and one reciprocal for both dens.
    order = [x for p_ in zip(range(8), range(8, 16)) for x in p_]
    out_ps = None
    for p in range(NGROUPS // 2):
        if p % 4 == 0:
            out_ps = pout.tile([128, D], f32, tag="out")
        ums = []
        for g in (2 * p, 2 * p + 1):
            # mask_rep[(il,h), j] = maskseg[16g + il, j], replicated over h
            # via a zero-stride DMA read dimension.
            mask_rep = maskp.tile([128, N], f16, tag="mask")
            in_ap = bass.AP(d_mask.tensor, (GSIZE * g) * N,
                            [[N, GSIZE], [0, H], [1, N]])
            nc.sync.dma_start(out=mask_rep[:], in_=in_ap)

            e_ps = ppe.tile([128, N], f32, tag="pe")
            # 0.2*alT linear term (same weights every group; rhs is g_lT)
            nc.tensor.matmul(e_ps[:], s_Arep[:], g_lT[:], start=True,
                             stop=False, skip_group_check=True)
            # visit halves alternately (0,8,1,9,...) so consecutive matmuls
            # hit different PE column strips and weight loads overlap compute
            for n_, k in enumerate(order):
                i = GSIZE * g + k  # maskseg row; g_rT column is the same i
                t_t = tpool.tile([H * F, N], f16, tag="t")
                if k not in (5, 10, 15):
                    # (g_lT + bias) max 0.0, one DVE pass (packed 16-bit mode)
                    nc.vector.tensor_scalar(t_t[:], g_lT[:], g_rT[:, i:i + 1],
                                            0.0, add, amax)
                else:
                    nc.scalar.activation(t_t[:], g_lT[:], Relu,
                                         bias=g_rT[:, i:i + 1], scale=1.0)
                half = 64 * (k // 8)
                w0 = 56 - 8 * (k % 8)
                nc.tensor.matmul(e_ps[half:half + 64, :],
                                 s_Aaw[:, w0:w0 + 64], t_t[:],
                                 start=False, stop=(n_ >= len(order) - 2),
                                 tile_position=(0, half),
                                 skip_group_check=True)

            u = upool.tile([128, N], f16, tag="u")
            nc.scalar.activation(u[:], e_ps[:], Exp)
            um = umpool.tile([128, N], f16, tag="um")
            nc.vector.tensor_mul(um[:], u[:], mask_rep[:])
            ums.append(um)

        # transpose both groups' um (4 chunks of 128 each) into one bank
        umt_ps = pumt.tile([128, 2 * N], f16, tag="umt")
        for gg in range(2):
            for c in range(4):
                cs = slice(128 * c, 128 * (c + 1))
                nc.tensor.transpose(umt_ps[:, 512 * gg + 128 * c:
                                           512 * gg + 128 * (c + 1)],
                                    ums[gg][:, cs], s_id[:])
        umt = umtp.tile([128, 2 * N], f16, tag="umtsb")
        nc.scalar.copy(umt[:], umt_ps[:])

        # aggT[(il,h), hf] = sum_j um[(il,h), j] * g_r[j, hf]; the ones
        # column of g_r_nat yields den = sum_j um in cols 128 / 257.
        agg_ps = pagg.tile([128, 258], f32, tag="agg")
        for gg in range(2):
            for c in range(4):
                nc.tensor.matmul(agg_ps[:, 129 * gg:129 * gg + 129],
                                 umt[:, 512 * gg + 128 * c:
                                        512 * gg + 128 * (c + 1)],
                                 g_r_nat[:, 129 * c:129 * c + 129],
                                 start=(c == 0), stop=(c == 3))
        rd2 = denp.tile([128, 2], f32, tag="rden")
        den_ap = bass.AP(agg_ps.tensor, agg_ps[:, 128:129].offset,
                         [agg_ps[:, 128:129].ap[0], [129, 2], [1, 1]])
        nc.vector.reciprocal(rd2[:], den_ap)

        for gg in range(2):
            g = 2 * p + gg
            # normalize rows by 1/den, keep only the matching head block
            agg_sb = aggp.tile([128, D], f16, tag="aggsb")
            nc.vector.scalar_tensor_tensor(
                agg_sb[:], agg_ps[:, 129 * gg:129 * gg + 128],
                rd2[:, gg:gg + 1], s_hm[:], mult, mult)

            # out[16q + il, hf] = sum_h agg_sb[(il,h), hf]; 4 groups
            # accumulate into a 64-row stripe via zero-masked weights.
            q = g % 4
            stripe = 64 * ((g % 8) // 4)
            nc.tensor.matmul(out_ps[stripe:stripe + 64, :],
                             s_R[:, 64 * q:64 * q + 64], agg_sb[:],
                             start=(q == 0), stop=(q == 3))

            if g % 4 == 3:
                outb = outp.tile([64, D], f32, tag="outb")
                nc.scalar.copy(outb[:], out_ps[stripe:stripe + 64, :])
                r0 = 64 * (g // 4)
                nc.sync.dma_start(out=d_out[r0:r0 + 64, :], in_=outb[:])


def _host_inputs(h, adj, W_l, W_r, a_w):
    """Build the per-core input maps (pure layout/constant prep)."""
    HF = H * F
    Aaw = np.zeros((HF, H), dtype=np.float32)
    for hh in range(H):
        Aaw[hh * F:(hh + 1) * F, hh] = a_w
    Amask = np.zeros((HF, 120), dtype=np.float32)
    Amask[:, 56:64] = (1.0 - NEG_SLOPE) * Aaw
    Amask = Amask.astype(np.float16)
    Arep02 = np.zeros((HF, 128), dtype=np.float32)
    for il in range(GSIZE):
        Arep02[:, il * H:(il + 1) * H] = NEG_SLOPE * Aaw
    Arep02 = Arep02.astype(np.float16)
    Rmask = np.zeros((128, 4 * 64), dtype=np.float16)
    for q in range(4):
        for il in range(GSIZE):
            Rmask[il * H:(il + 1) * H, 64 * q + 16 * q + il] = 1.0
    headmask = np.zeros((128, HF), dtype=np.float32)
    for il in range(GSIZE):
        for hh in range(H):
            headmask[il * H + hh, hh * F:(hh + 1) * F] = 1.0
    ident = np.eye(128, dtype=np.float16)
    WlT = np.ascontiguousarray(W_l.T).astype(np.float16)
    WrT = np.ascontiguousarray(W_r.T).astype(np.float16)

    in_maps = []
    for c in range(NCORES):
        b = c // 2
        i0 = IHALF * (c % 2)
        # Roll the node axis so this core's target nodes sit at positions
        # 0..IHALF-1 (the SPMD program indexes g_rT bias columns by local i).
        # Source-node order is permuted consistently everywhere (softmax and
        # aggregation are permutation-invariant over j).
        in_maps.append({
            "hT": np.ascontiguousarray(np.roll(h[b], -i0, axis=0).T).astype(
                np.float16),
            "WlT": WlT,
            "WrT": WrT,
            "Amask": Amask,
            "Arep02": Arep02,
            "Rmask": Rmask,
            "headmask": headmask,
            "ident": ident,
            "maskseg": np.ascontiguousarray(np.roll(
                adj[b, i0:i0 + IHALF, :, 0], -i0, axis=1)).astype(np.float16),
        })
    return in_maps


_NC_CACHE = {}
LAST_RESULT = None  # BassKernelResults of the most recent kernel() call


def _get_program():
    if "nc" not in _NC_CACHE:
        _NC_CACHE["nc"] = build_program()
    return _NC_CACHE["nc"]


def kernel(h, adj, W_l, W_r, a_w):
    h = np.asarray(h)
    adj = np.asarray(adj)
    W_l = np.asarray(W_l)
    W_r = np.asarray(W_r)
    a_w = np.asarray(a_w)

    nc = _get_program()
    in_maps = _host_inputs(h, adj, W_l, W_r, a_w)
    res = None
    for attempt in range(3):
        try:
            res = run_bass_kernel_spmd(nc, in_maps, list(range(NCORES)))
            break
        except Exception:
            # the axon-proxied device occasionally reports a transient
            # "unrecoverable" state at process start; it self-heals
            if attempt == 2:
                raise
            import time
            time.sleep(20)
    global LAST_RESULT
    LAST_RESULT = res

    out = np.zeros((B, N, D), dtype=np.float32)
    for c in range(NCORES):
        b = c // 2
        i0 = IHALF * (c % 2)
        out[b, i0:i0 + IHALF, :] = res.results[c]["out"]
    return out


# revision 59
# speedup vs baseline: 1.0761x; 1.0761x over previous
"""GATv2 layer kernel for Trainium2 (Bass/Tile), 8-core SPMD.

Problem (hardcoded): B=4, N=512, D=128, H=8 heads, F=16 hidden, is_concat.
  g_l = h @ W_l.T ; g_r = h @ W_r.T               [B,N,H,F]
  e[b,i,j,h] = sum_f a_w[f]*lrelu(g_l[b,j,h,f] + g_r[b,i,h,f], 0.2)
  a = softmax_j(e masked by adj)                  [B,i,j,H]
  out[b,i,h,f] = sum_j a[b,i,j,h]*g_r[b,j,h,f]   -> [B,N,H*F]

Sharding: 8 cores = (batch b in 0..4) x (i-half in 0..2). Each core handles
256 target nodes i of one batch with fully-replicated g_l/g_r.

Math used on device (per core, b fixed):
  lrelu(x) = 0.8*relu(x) + 0.2*x, so
  e[i,j,h] = 0.8*sum_hf A[hf,h]*relu(g_lT[hf,j] + g_rT[hf,i]) + 0.2*alT[h,j]
             + 0.2*ar[i,h]
  The per-(i,h) additive term 0.2*ar cancels in softmax (shift invariance),
  so it is dropped. Masking is multiplicative on exp(e) (exact zeros).
  Softmax normalization is applied after aggregation (linearity).

Layouts (per group of 16 i's, partitions = (i_local*8 + h)):
  t[hf, j]      = relu(g_lT + g_rT[:, i] bias)        ACT/DVE
  psum[(i,h),j] = A_aw.T @ t  (M=8 stripes)           PE
  psum += 0.2*alT via an extra accumulating matmul   PE
  u             = exp(psum)                           ACT (reads PSUM)
  um, den       = u * mask_rep, rowsum                DVE (fused)
  umT           = transpose(um) (4x 128x128)          PE
  aggT[(i,h),hf]= sum_j umT.T @ g_r                   PE (4 K-chunks)
  agg_sb        = aggT * (1/den) * headmask           DVE (fused)
  out[i,hf]     = R.T @ agg_sb  (sum over h)          PE
"""

import ml_dtypes
import numpy as np
from contextlib import ExitStack

import concourse.bass as bass
import concourse.bacc as bacc
import concourse.tile as tile
import concourse.mybir as mybir
from concourse.bass_utils import run_bass_kernel_spmd

B, N, D = 4, 512, 128
H, F = 8, 16
NEG_SLOPE = 0.2
NCORES = 8
IHALF = N // 2          # 256 target nodes per core
GSIZE = 16              # i's per group
NGROUPS = IHALF // GSIZE  # 16
f32 = mybir.dt.float32
f16 = mybir.dt.float16

# The score path (relu'd pairwise features t and the per-node score matmuls)
# runs in fp16: full 1-cycle/row PE streaming, DVE packed 16-bit modes, and
# fast-weight-load with background-buffer overlap -- with a 10-bit mantissa
# (TF32-class, ~5e-4) and ample range for these tiny values. The noise lands
# only in pre-softmax scores; softmax normalization and the aggregation path
# (attention weights x g_r and the output) stay exact fp32.

# How many of the 16 per-group relu ops go to DVE (rest on ScalarE).
RELU_ON_DVE = 12


def build_program():
    nc = bacc.Bacc(
        "TRN2", target_bir_lowering=False, debug=False, num_devices=NCORES
    )

    d_hT = nc.dram_tensor("hT", [D, N], f16, kind="ExternalInput").ap()
    d_WlT = nc.dram_tensor("WlT", [D, H * F], f16, kind="ExternalInput").ap()
    d_WrT = nc.dram_tensor("WrT", [D, H * F], f16, kind="ExternalInput").ap()
    # Amask[:, 64k:64k+64] is 0.8*A_aw placed in the (k%8)-th 8-column block
    # of a [128, 64] half-array stationary operand (zeros elsewhere): 16
    # accumulating M=64 matmuls (8 per 64-row half, alternating PE column
    # strips so LDWEIGHTS overlaps the other half's in-flight matmul) compose
    # 16 target nodes into one full-height PSUM tile. lrelu = 0.8*relu +
    # 0.2*identity; the 0.8 is folded into these weights.
    d_Aaw = nc.dram_tensor("Amask", [H * F, 120], f16, kind="ExternalInput").ap()
    d_Arep = nc.dram_tensor("Arep02", [H * F, 128], f16, kind="ExternalInput").ap()
    # Rmask[:, 64q:64q+64] holds the head-sum reduction matrix placed in
    # columns [16q:16q+16] (4 accumulating matmuls -> one 64-row PSUM stripe).
    d_R = nc.dram_tensor("Rmask", [128, 4 * 64], f16, kind="ExternalInput").ap()
    d_hm = nc.dram_tensor("headmask", [128, H * F], f32, kind="ExternalInput").ap()
    d_id = nc.dram_tensor("ident", [128, 128], f16, kind="ExternalInput").ap()
    d_mask = nc.dram_tensor("maskseg", [IHALF, N], f16, kind="ExternalInput").ap()
    d_out = nc.dram_tensor("out", [IHALF, D], f32, kind="ExternalOutput").ap()

    with tile.TileContext(nc) as tc:
        with ExitStack() as ctx:
            _gat_body(ctx, tc, d_out, d_hT, d_WlT, d_WrT, d_Aaw, d_Arep,
                      d_R, d_hm, d_id, d_mask)
    nc.compile()
    return nc


def _gat_body(ctx, tc, d_out, d_hT, d_WlT, d_WrT, d_Aaw, d_Arep, d_R, d_hm,
              d_id, d_mask):
    nc = tc.nc
    add = mybir.AluOpType.add
    mult = mybir.AluOpType.mult
    amax = mybir.AluOpType.max
    Relu = mybir.ActivationFunctionType.Relu
    Exp = mybir.ActivationFunctionType.Exp

    consts = ctx.enter_context(tc.tile_pool(name="consts", bufs=1))
    tpool = ctx.enter_context(tc.tile_pool(name="tpool", bufs=12))
    upool = ctx.enter_context(tc.tile_pool(name="upool", bufs=3))
    umpool = ctx.enter_context(tc.tile_pool(name="umpool", bufs=4))
    maskp = ctx.enter_context(tc.tile_pool(name="maskp", bufs=4))
    umtp = ctx.enter_context(tc.tile_pool(name="umtp", bufs=3))
    aggp = ctx.enter_context(tc.tile_pool(name="aggp", bufs=3))
    denp = ctx.enter_context(tc.tile_pool(name="denp", bufs=3))
    outp = ctx.enter_context(tc.tile_pool(name="outp", bufs=2))

    ppe = ctx.enter_context(tc.tile_pool(name="ppe", bufs=3, space="PSUM"))
    pumt = ctx.enter_context(tc.tile_pool(name="pumt", bufs=2, space="PSUM"))
    pagg = ctx.enter_context(tc.tile_pool(name="pagg", bufs=2, space="PSUM"))
    pout = ctx.enter_context(tc.tile_pool(name="pout", bufs=1, space="PSUM"))

    # ---- load constants ----
    s_WlT = consts.tile([D, H * F], f16, tag="wlt")
    nc.sync.dma_start(out=s_WlT[:], in_=d_WlT)
    s_WrT = consts.tile([D, H * F], f16, tag="wrt")
    nc.sync.dma_start(out=s_WrT[:], in_=d_WrT)
    s_hT = consts.tile([D, N], f16, tag="ht")
    nc.sync.dma_start(out=s_hT[:], in_=d_hT)
    s_Aaw = consts.tile([H * F, 120], f16, tag="aaw")
    nc.gpsimd.dma_start(out=s_Aaw[:], in_=d_Aaw)
    s_Arep = consts.tile([H * F, 128], f16, tag="arep")
    nc.gpsimd.dma_start(out=s_Arep[:], in_=d_Arep)
    s_R = consts.tile([128, 4 * 64], f16, tag="rmat")
    nc.gpsimd.dma_start(out=s_R[:], in_=d_R)
    s_hm = consts.tile([128, H * F], f32, tag="hm")
    nc.gpsimd.dma_start(out=s_hm[:], in_=d_hm)
    s_id = consts.tile([128, 128], f16, tag="ident")
    nc.gpsimd.dma_start(out=s_id[:], in_=d_id)

    # ---- setup: projections ----
    # g_lT[hf, j] = sum_d WlT[d, hf] * hT[d, j]  (kept in bf16: feeds the
    # bf16 score path only)
    g_lT = consts.tile([H * F, N], f16, tag="glt")
    ps = ppe.tile([128, N], f32, tag="pe")
    nc.tensor.matmul(ps[:], s_WlT[:], s_hT[:], start=True, stop=True)
    nc.scalar.copy(g_lT[:], ps[:])

    g_rT = consts.tile([H * F, N], f32, tag="grt")
    ps = ppe.tile([128, N], f32, tag="pe")
    nc.tensor.matmul(ps[:], s_WrT[:], s_hT[:], start=True, stop=True)
    nc.scalar.copy(g_rT[:], ps[:])

    # g_r natural layout with an appended ones column per 128-row chunk:
    # chunk c occupies cols [129c, 129c+129); col 129c+128 is all-ones so the
    # aggregation matmul also produces the softmax denominator for free.
    g_r_nat = consts.tile([128, 4 * 129], f16, tag="grnat")
    for c in range(4):
        cs = slice(128 * c, 128 * (c + 1))
        pq = pagg.tile([128, 129], f32, tag="agg")
        nc.tensor.matmul(pq[:, 0:128], s_hT[:, cs], s_WrT[:],
                         start=True, stop=True)
        nc.vector.tensor_copy(g_r_nat[:, 129 * c:129 * c + 128], pq[:, 0:128])
        nc.vector.memset(g_r_nat[:, 129 * c + 128:129 * c + 129], 1.0)

    # The 0.2*alT linear term is accumulated into each group's score PSUM by
    # an extra matmul (lhsT=s_Arep, rhs=g_lT) -- no materialized alT tile.

    # ---- main loop: pairs of 16-node groups ----
    # The transpose/aggregation stage is batched per pair: one umT PSUM bank
    # (fp16 [128,1024] fits a 2KiB bank), one PSUM->SBUF copy, one paired
    # aggregation PSUM tile [128,258] and one reciprocal for both dens.
    order = [x for p_ in zip(range(8), range(8, 16)) for x in p_]
    out_ps = None
    for p in range(NGROUPS // 2):
        if p % 4 == 0:
            out_ps = pout.tile([128, D], f32, tag="out")
        ums = []
        for g in (2 * p, 2 * p + 1):
            # mask_rep[(il,h), j] = maskseg[16g + il, j], replicated over h
            # via a zero-stride DMA read dimension.
            mask_rep = maskp.tile([128, N], f16, tag="mask")
            in_ap = bass.AP(d_mask.tensor, (GSIZE * g) * N,
                            [[N, GSIZE], [0, H], [1, N]])
            nc.sync.dma_start(out=mask_rep[:], in_=in_ap)

            e_ps = ppe.tile([128, N], f32, tag="pe")
            # 0.2*alT linear term (same weights every group; rhs is g_lT)
            nc.tensor.matmul(e_ps[:], s_Arep[:], g_lT[:], start=True,
                             stop=False, skip_group_check=True)
            # visit halves alternately (0,8,1,9,...) so consecutive matmuls
            # hit different PE column strips and weight loads overlap compute
            for n_, k in enumerate(order):
                i = GSIZE * g + k  # maskseg row; g_rT column is the same i
                t_t = tpool.tile([H * F, N], f16, tag="t")
                if k not in (2, 5, 8, 11, 14):
                    # (g_lT + bias) max 0.0, one DVE pass (packed 16-bit mode)
                    nc.vector.tensor_scalar(t_t[:], g_lT[:], g_rT[:, i:i + 1],
                                            0.0, add, amax)
                else:
                    nc.scalar.activation(t_t[:], g_lT[:], Relu,
                                         bias=g_rT[:, i:i + 1], scale=1.0)
                half = 64 * (k // 8)
                w0 = 56 - 8 * (k % 8)
                nc.tensor.matmul(e_ps[half:half + 64, :],
                                 s_Aaw[:, w0:w0 + 64], t_t[:],
                                 start=False, stop=(n_ >= len(order) - 2),
                                 tile_position=(0, half),
                                 skip_group_check=True)

            u = upool.tile([128, N], f16, tag="u")
            nc.scalar.activation(u[:], e_ps[:], Exp)
            um = umpool.tile([128, N], f16, tag="um")
            nc.vector.tensor_mul(um[:], u[:], mask_rep[:])
            ums.append(um)

        # transpose both groups' um (4 chunks of 128 each) into one bank
        umt_ps = pumt.tile([128, 2 * N], f16, tag="umt")
        for gg in range(2):
            for c in range(4):
                cs = slice(128 * c, 128 * (c + 1))
                nc.tensor.transpose(umt_ps[:, 512 * gg + 128 * c:
                                           512 * gg + 128 * (c + 1)],
                                    ums[gg][:, cs], s_id[:])
        umt = umtp.tile([128, 2 * N], f16, tag="umtsb")
        nc.scalar.copy(umt[:], umt_ps[:])

        # aggT[(il,h), hf] = sum_j um[(il,h), j] * g_r[j, hf]; the ones
        # column of g_r_nat yields den = sum_j um in cols 128 / 257.
        agg_ps = pagg.tile([128, 258], f32, tag="agg")
        for gg in range(2):
            for c in range(4):
                nc.tensor.matmul(agg_ps[:, 129 * gg:129 * gg + 129],
                                 umt[:, 512 * gg + 128 * c:
                                        512 * gg + 128 * (c + 1)],
                                 g_r_nat[:, 129 * c:129 * c + 129],
                                 start=(c == 0), stop=(c == 3))
        rd2 = denp.tile([128, 2], f32, tag="rden")
        den_ap = bass.AP(agg_ps.tensor, agg_ps[:, 128:129].offset,
                         [agg_ps[:, 128:129].ap[0], [129, 2], [1, 1]])
        nc.vector.reciprocal(rd2[:], den_ap)

        for gg in range(2):
            g = 2 * p + gg
            # normalize rows by 1/den, keep only the matching head block
            agg_sb = aggp.tile([128, D], f16, tag="aggsb")
            nc.vector.scalar_tensor_tensor(
                agg_sb[:], agg_ps[:, 129 * gg:129 * gg + 128],
                rd2[:, gg:gg + 1], s_hm[:], mult, mult)

            # out[16q + il, hf] = sum_h agg_sb[(il,h), hf]; 4 groups
            # accumulate into a 64-row stripe via zero-masked weights.
            q = g % 4
            stripe = 64 * ((g % 8) // 4)
            nc.tensor.matmul(out_ps[stripe:stripe + 64, :],
                             s_R[:, 64 * q:64 * q + 64], agg_sb[:],
                             start=(q == 0), stop=(q == 3))

            if g % 4 == 3:
                outb = outp.tile([64, D], f32, tag="outb")
                nc.scalar.copy(outb[:], out_ps[stripe:stripe + 64, :])
                r0 = 64 * (g // 4)
                nc.sync.dma_start(out=d_out[r0:r0 + 64, :], in_=outb[:])


def _host_inputs(h, adj, W_l, W_r, a_w):
    """Build the per-core input maps (pure layout/constant prep)."""
    HF = H * F
    Aaw = np.zeros((HF, H), dtype=np.float32)
    for hh in range(H):
        Aaw[hh * F:(hh + 1) * F, hh] = a_w
    Amask = np.zeros((HF, 120), dtype=np.float32)
    Amask[:, 56:64] = (1.0 - NEG_SLOPE) * Aaw
    Amask = Amask.astype(np.float16)
    Arep02 = np.zeros((HF, 128), dtype=np.float32)
    for il in range(GSIZE):
        Arep02[:, il * H:(il + 1) * H] = NEG_SLOPE * Aaw
    Arep02 = Arep02.astype(np.float16)
    Rmask = np.zeros((128, 4 * 64), dtype=np.float16)
    for q in range(4):
        for il in range(GSIZE):
            Rmask[il * H:(il + 1) * H, 64 * q + 16 * q + il] = 1.0
    headmask = np.zeros((128, HF), dtype=np.float32)
    for il in range(GSIZE):
        for hh in range(H):
            headmask[il * H + hh, hh * F:(hh + 1) * F] = 1.0
    ident = np.eye(128, dtype=np.float16)
    WlT = np.ascontiguousarray(W_l.T).astype(np.float16)
    WrT = np.ascontiguousarray(W_r.T).astype(np.float16)

    in_maps = []
    for c in range(NCORES):
        b = c // 2
        i0 = IHALF * (c % 2)
        # Roll the node axis so this core's target nodes sit at positions
        # 0..IHALF-1 (the SPMD program indexes g_rT bias columns by local i).
        # Source-node order is permuted consistently everywhere (softmax and
        # aggregation are permutation-invariant over j).
        in_maps.append({
            "hT": np.ascontiguousarray(np.roll(h[b], -i0, axis=0).T).astype(
                np.float16),
            "WlT": WlT,
            "WrT": WrT,
            "Amask": Amask,
            "Arep02": Arep02,
            "Rmask": Rmask,
            "headmask": headmask,
            "ident": ident,
            "maskseg": np.ascontiguousarray(np.roll(
                adj[b, i0:i0 + IHALF, :, 0], -i0, axis=1)).astype(np.float16),
        })
    return in_maps


_NC_CACHE = {}
LAST_RESULT = None  # BassKernelResults of the most recent kernel() call


def _get_program():
    if "nc" not in _NC_CACHE:
        _NC_CACHE["nc"] = build_program()
    return _NC_CACHE["nc"]


def kernel(h, adj, W_l, W_r, a_w):
    h = np.asarray(h)
    adj = np.asarray(adj)
    W_l = np.asarray(W_l)
    W_r = np.asarray(W_r)
    a_w = np.asarray(a_w)

    nc = _get_program()
    in_maps = _host_inputs(h, adj, W_l, W_r, a_w)
    res = None
    for attempt in range(3):
        try:
            res = run_bass_kernel_spmd(nc, in_maps, list(range(NCORES)))
            break
        except Exception:
            # the axon-proxied device occasionally reports a transient
            # "unrecoverable" state at process start; it self-heals
            if attempt == 2:
                raise
            import time
            time.sleep(20)
    global LAST_RESULT
    LAST_RESULT = res

    out = np.zeros((B, N, D), dtype=np.float32)
    for c in range(NCORES):
        b = c // 2
        i0 = IHALF * (c % 2)
        out[b, i0:i0 + IHALF, :] = res.results[c]["out"]
    return out
